# revision 2
# baseline (speedup 1.0000x reference)
"""Trainium2 8-core kernel for causal GQA attention (nn_Attention_90967407329949).

Distribution: 2-way tensor-parallel over head groups x 4-way data-parallel over
batch. Core c = (b = c//2, g = c%2) computes batch b with q heads g*8..g*8+7
and kv heads 2g, 2g+1 (no KV-projection redundancy). After attention, the
pair {2b, 2b+1} exchanges attention outputs via two 2-rank AllGathers (one per
512-token block); each core computes the output-projection column slice
out[:, g*1024:(g+1)*1024] for its batch. Host concatenates slices.

Causal wavefront pipeline per core (emission order = engine queue order):
  A: QKV projections for token block 0
  B: attention on query block 0 (needs only keys 0..511) interleaved with
     QKV projections for token block 1
  C: exchange #1 dispatched; attention on query block 1
  D: output projection block 0 (exchange #1 lands mid-C); exchange #2
  E: output projection block 1

All matmuls bf16 with fp32 PSUM. Softmax skips the running max (|s|max ~ 7).
The denominator reciprocal is broadcast across partitions with a K=1 TensorE
matmul (ones_row^T @ recip) instead of gpsimd.partition_broadcast, so the
GpSimd queue carries only the two collectives and never stalls attention.
The output projection reads the full gathered attention (own half included)
from ag_out so the SPMD program needs no per-core row offsets.

Layouts (feature dim on SBUF partitions):
  xt   [2048, 1024]   x[b]^T, bf16; on-chip [128, 16, 1024] k-tiles
  wq   [128, 16, 1024] Wq k-tiles for the 8 q heads of group g
  wk/wv[128, 16, 256]  k-tiles for the 2 kv heads of group g
  wo   [128, 16, 1024] (head_scale-folded) Wo k-tiles, column slice g
  cost/sint [128, 1024] rotary tables transposed; sint sign-folded
  mask [128, 2048]     4 causal masks for the 4 diagonal offsets
  out  [1024, 1024]    (out @ Wo)^T column slice, bf16 (host upcasts)
"""

import numpy as np
import ml_dtypes

import concourse.bacc as bacc
import concourse.mybir as mybir
import concourse.tile as tile
from concourse.bass_utils import run_bass_kernel_spmd

BF16 = mybir.dt.bfloat16
F32 = mybir.dt.float32

N_CORES = 8
B = 4
N = 1024           # sequence length per batch (per-core tokens)
D = 2048           # model dim
DH = 128           # head dim
KT = D // 128      # 16 contraction k-tiles
NQH = 8            # q heads per core
SCALE = 1.0 / np.sqrt(DH)

_NC_CACHE = {}


def build_nc():
    if "nc" in _NC_CACHE:
        return _NC_CACHE["nc"]
    nc = bacc.Bacc("TRN2", target_bir_lowering=False, debug=False,
                   num_devices=N_CORES)

    xt = nc.dram_tensor("xt", [D, N], BF16, kind="ExternalInput")
    wq = nc.dram_tensor("wq", [128, NQH, KT, 128], BF16, kind="ExternalInput")
    wk = nc.dram_tensor("wk", [128, KT, 256], BF16, kind="ExternalInput")
    wv = nc.dram_tensor("wv", [128, KT, 256], BF16, kind="ExternalInput")
    wo = nc.dram_tensor("wo", [128, KT, 1024], BF16, kind="ExternalInput")
    cost = nc.dram_tensor("cost", [128, N], BF16, kind="ExternalInput")
    sint = nc.dram_tensor("sint", [128, N], BF16, kind="ExternalInput")
    mask = nc.dram_tensor("mask", [128, 2048], BF16, kind="ExternalInput")
    out = nc.dram_tensor("out", [1024, N], BF16, kind="ExternalOutput")

    # Pair-exchange buffers: one per (token block, head half) so each
    # collective can start as soon as its 4 heads finish (tensor-granular
    # deps). Half q holds local heads 4q..4q+3; the gather output rows are
    # [rank0 h, rank1 h] = global heads [4q..4q+3, 8+4q..8+4q+3]; the host
    # reorders wo k-tiles to match.
    pair_groups = [[0, 1], [2, 3], [4, 5], [6, 7]]
    # Uneven 6/2 head split: the trailing exchange (heads 6-7) is small
    # so the end-of-attention -> output-projection tail stays short.
    HSPLIT = 4
    ag_in = [[nc.dram_tensor(f"ag_in{p}_{q}",
                             [(HSPLIT if q == 0 else NQH - HSPLIT) * 128, 512],
                             BF16) for q in range(2)] for p in range(2)]
    ag_out = [[nc.dram_tensor(f"ag_out{p}_{q}",
                              [2 * (HSPLIT if q == 0 else NQH - HSPLIT) * 128,
                               512], BF16) for q in range(2)] for p in range(2)]

    with tile.TileContext(nc) as tc:
        with (
            tc.tile_pool(name="const", bufs=1) as constp,
            tc.tile_pool(name="persist", bufs=1) as persist,
            tc.tile_pool(name="qkraw", bufs=3) as qkrawp,
            tc.tile_pool(name="rope", bufs=2) as ropep,
            tc.tile_pool(name="attp", bufs=3) as attp,
            tc.tile_pool(name="ep", bufs=4) as ep,
            tc.tile_pool(name="etmpp", bufs=2) as etmpp,
            tc.tile_pool(name="recipp", bufs=2) as recipp,
            tc.tile_pool(name="rbcp", bufs=2) as rbcp,
            tc.tile_pool(name="gp", bufs=2) as gp,
            tc.tile_pool(name="oobp", bufs=2) as oobp,
            tc.tile_pool(name="psacc", bufs=2, space="PSUM") as psacc,
            tc.tile_pool(name="pss", bufs=2, space="PSUM") as pss,
            tc.tile_pool(name="psu", bufs=2, space="PSUM") as psu,
            tc.tile_pool(name="pssum", bufs=1, space="PSUM") as pssum,
            tc.tile_pool(name="psb", bufs=1, space="PSUM") as psb,
        ):
            # ---- constants / persistent ----
            wq_sb = constp.tile([128, NQH, KT, 128], BF16)
            wk_sb = constp.tile([128, KT, 256], BF16)
            wv_sb = constp.tile([128, KT, 256], BF16)
            wo_sb = constp.tile([128, KT, 1024], BF16)
            cos_sb = constp.tile([128, N], BF16)
            sin_sb = constp.tile([128, N], BF16)
            mask_sb = constp.tile([128, 2048], BF16)
            ones_sb = constp.tile([128, 1], BF16)
            ones_row = constp.tile([1, 128], BF16)
            xf = persist.tile([128, KT, N], BF16)

            xt_r = xt.rearrange("(t p) n -> p t n", p=128)

            # Bulk loads ride the gpsimd (SWDGE) and scalar rings so the
            # sync ring stays lean for small latency-critical transfers
            # (rope rotations, v transposes, exchange staging). First-use
            # order: K-proj needs wk + xf block 0, V-proj wv, then wq heads.
            nc.gpsimd.dma_start(wk_sb[:, 0:8], wk[:, 0:8])
            nc.scalar.dma_start(wv_sb[:, 0:8], wv[:, 0:8])
            nc.gpsimd.dma_start(xf[:, 0:8, 0:512], xt_r[:, 0:8, 0:512])
            nc.scalar.dma_start(xf[:, 8:16, 0:512], xt_r[:, 8:16, 0:512])
            nc.gpsimd.dma_start(wk_sb[:, 8:16], wk[:, 8:16])
            nc.scalar.dma_start(wv_sb[:, 8:16], wv[:, 8:16])
            nc.sync.dma_start(cos_sb[:], cost[:])
            nc.sync.dma_start(sin_sb[:], sint[:])
            for h in range(NQH):
                ring = nc.gpsimd if h < 4 else nc.scalar
                ring.dma_start(wq_sb[:, h], wq[:, h])
            nc.sync.dma_start(mask_sb[:], mask[:])
            nc.vector.memset(ones_sb[:], 1.0)
            nc.vector.memset(ones_row[:], 1.0)

            def late_consts():
                # xf block 1, then wo (needed last)
                nc.gpsimd.dma_start(xf[:, 0:8, 512:1024],
                                    xt_r[:, 0:8, 512:1024])
                nc.scalar.dma_start(xf[:, 8:16, 512:1024],
                                    xt_r[:, 8:16, 512:1024])
                nc.gpsimd.dma_start(wo_sb[:], wo[:])

            # ---- persistent per-core QKV (RoPE'd, transposed layouts) ----
            q_sb = [persist.tile([128, N], BF16, name=f"q{h}_sb")
                    for h in range(NQH)]
            k_sb = [persist.tile([128, N], BF16, name=f"k{kv}_sb")
                    for kv in range(2)]
            v_sb = [persist.tile([128, N], BF16, name=f"v{kv}_sb")
                    for kv in range(2)]

            def rope_chunk(raw, dst, nb):
                """RoPE 512 positions (block nb) from raw into dst."""
                c0 = nb * 512
                rot = ropep.tile([128, 512], BF16, tag="rot")
                nc.sync.dma_start(rot[0:64, :], raw[64:128, :])
                nc.sync.dma_start(rot[64:128, :], raw[0:64, :])
                t1 = ropep.tile([128, 512], BF16, tag="t1")
                nc.vector.tensor_mul(t1[:], raw[:], cos_sb[:, c0:c0 + 512])
                t2 = ropep.tile([128, 512], BF16, tag="t2")
                nc.vector.tensor_mul(t2[:], rot[:], sin_sb[:, c0:c0 + 512])
                nc.vector.tensor_add(dst[:, c0:c0 + 512], t1[:], t2[:])

            def qkv_blk(nb):
                """Projections + RoPE for token block nb (512 tokens)."""
                col0 = nb * 512
                if nb == 1:
                    late_consts()
                holder = {}

                def accum(w_sb, msl, name):
                    ps = psacc.tile([128, 512], F32, tag="psacc", name=name)
                    for k0 in range(0, KT, 4):
                        for kt in range(k0, k0 + 4):
                            nc.tensor.matmul(
                                ps[:], w_sb[:, kt, msl],
                                xf[:, kt, col0:col0 + 512],
                                start=(kt == 0), stop=(kt == KT - 1))
                        yield
                    holder["ps"] = ps

                # K, V first (attention consumes them head-by-head)
                for kv in range(2):
                    msl = slice(kv * 128, (kv + 1) * 128)
                    yield from accum(wk_sb, msl, f"k_ps_{nb}_{kv}")
                    kraw = qkrawp.tile([128, 512], BF16, tag="raw")
                    nc.scalar.activation(kraw[:], holder["ps"][:],
                                         mybir.ActivationFunctionType.Copy)
                    rope_chunk(kraw, k_sb[kv], nb)
                    yield
                    yield from accum(wv_sb, msl, f"v_ps_{nb}_{kv}")
                    vraw = qkrawp.tile([128, 512], BF16, tag="raw")
                    nc.scalar.activation(vraw[:], holder["ps"][:],
                                         mybir.ActivationFunctionType.Copy)
                    for i in range(4):
                        ring = nc.sync if i % 2 == 0 else nc.scalar
                        ring.dma_start_transpose(
                            v_sb[kv][:, col0 + i * 128:col0 + (i + 1) * 128],
                            vraw[:, i * 128:(i + 1) * 128])
                    yield
                for h in range(NQH):
                    yield from accum(wq_sb[:, h], slice(0, 128),
                                     f"q_ps_{nb}_{h}")
                    qraw = qkrawp.tile([128, 512], BF16, tag="raw")
                    nc.scalar.activation(qraw[:], holder["ps"][:],
                                         mybir.ActivationFunctionType.Copy)
                    rope_chunk(qraw, q_sb[h], nb)
                    yield

            def att_ib(ib):
                """Attention for query block ib, all heads; yields per j-tile."""
                icol = ib * 512
                cnt = 4 * ib + 4
                for h in range(NQH):
                    qh = q_sb[h]
                    kv = h // 4
                    u_ps = psu.tile([128, 512], F32, tag="psu",
                                    name=f"u_ps_{ib}_{h}")
                    sum_ps = pssum.tile([1, 512], F32, tag="pssum",
                                        name=f"sum_ps_{ib}_{h}")

                    def c_lo(jt):
                        r = jt - 4 * ib
                        return 128 * r if r > 0 else 0

                    def s_mm(jt):
                        s_ps = pss.tile([128, 512], F32, tag="pss",
                                        name=f"s_ps_{ib}_{h}_{jt}")
                        jcol = jt * 128
                        c0 = c_lo(jt)
                        nc.tensor.matmul(
                            s_ps[:, c0:512], k_sb[kv][:, jcol:jcol + 128],
                            qh[:, icol + c0:icol + 512],
                            start=True, stop=True)
                        return s_ps

                    def e_of(jt, s_ps):
                        r = jt - 4 * ib
                        c0 = c_lo(jt)
                        e = ep.tile([128, 512], BF16, tag="e",
                                    name=f"e_{ib}_{h}_{jt}")
                        if r >= 0:  # diagonal tile: mask after exp
                            etmp = etmpp.tile([128, 512], BF16, tag="etmp")
                            nc.scalar.activation(
                                etmp[:, c0:512], s_ps[:, c0:512],
                                mybir.ActivationFunctionType.Exp, scale=SCALE)
                            nc.vector.tensor_mul(
                                e[:, c0:512], etmp[:, c0:512],
                                mask_sb[:, r * 512 + c0:(r + 1) * 512])
                        else:
                            nc.scalar.activation(
                                e[:], s_ps[:],
                                mybir.ActivationFunctionType.Exp, scale=SCALE)
                        return e

                    s_tiles = {0: s_mm(0), 1: s_mm(1)}
                    for jt in range(cnt):
                        e = e_of(jt, s_tiles.pop(jt))
                        if jt + 2 < cnt:
                            s_tiles[jt + 2] = s_mm(jt + 2)
                        c0 = c_lo(jt)
                        nc.tensor.matmul(
                            u_ps[:, c0:512],
                            v_sb[kv][:, jt * 128:(jt + 1) * 128], e[:, c0:512],
                            start=(jt == 0), stop=(jt == cnt - 1),
                            skip_group_check=True)
                        nc.tensor.matmul(
                            sum_ps[:, c0:512], ones_sb[:], e[:, c0:512],
                            start=(jt == 0), stop=(jt == cnt - 1),
                            skip_group_check=True)
                        yield
                    recip_f = recipp.tile([1, 512], F32, tag="recipf")
                    nc.vector.reciprocal_approx_fast(out=recip_f[:],
                                                     in_=sum_ps[:])
                    recip = recipp.tile([1, 512], BF16, tag="recip")
                    nc.vector.tensor_copy(recip[:], recip_f[:])
                    rbc_ps = psb.tile([128, 512], F32, tag="psb",
                                      name=f"rbc_ps_{ib}_{h}")
                    nc.tensor.matmul(rbc_ps[:], ones_row[:], recip[:],
                                     start=True, stop=True)
                    rbc = rbcp.tile([128, 512], BF16, tag="rbc")
                    nc.scalar.activation(rbc[:], rbc_ps[:],
                                         mybir.ActivationFunctionType.Copy)
                    atile = attp.tile([128, 512], BF16, tag="att",
                                      name=f"att_{ib}_{h}")
                    nc.vector.tensor_mul(atile[:], u_ps[:], rbc[:])
                    yield
                    ring = nc.sync if h % 2 == 0 else nc.scalar
                    q, hq = (0, h) if h < HSPLIT else (1, h - HSPLIT)
                    ring.dma_start(
                        ag_in[ib][q][hq * 128:(hq + 1) * 128, :], atile[:])

            def exchange(ib, q):
                nc.gpsimd.collective_compute(
                    "AllGather", mybir.AluOpType.bypass,
                    replica_groups=pair_groups,
                    ins=[ag_in[ib][q][:].opt()],
                    outs=[ag_out[ib][q][:].opt()])

            ag_out_r = [[t.rearrange("(t p) n -> p t n", p=128) for t in row]
                        for row in ag_out]

            def oproj_blk(ib):
                """Output projection for token block ib from the full gather."""
                icol = ib * 512
                n0, n1 = 2 * HSPLIT, 2 * (NQH - HSPLIT)
                g0 = gp.tile([128, n0, 512], BF16, tag="g0", name=f"g0_{ib}")
                g1 = gp.tile([128, n1, 512], BF16, tag="g1", name=f"g1_{ib}")
                nc.sync.dma_start(g0[:], ag_out_r[ib][0][:, 0:n0, :])
                nc.scalar.dma_start(g1[:], ag_out_r[ib][1][:, 0:n1, :])

                def gsrc(kt):
                    return g0[:, kt, :] if kt < n0 else g1[:, kt - n0, :]

                for m in range(8):
                    o_ps = psacc.tile([128, 512], F32, tag="psacc",
                                      name=f"o_ps_{ib}_{m}")
                    for k0 in range(0, KT, 4):
                        for kt in range(k0, k0 + 4):
                            nc.tensor.matmul(
                                o_ps[:], wo_sb[:, kt, m * 128:(m + 1) * 128],
                                gsrc(kt), start=(kt == 0),
                                stop=(kt == KT - 1))
                        yield
                    osb = oobp.tile([128, 512], BF16, tag="osb",
                                    name=f"osb_{ib}_{m}")
                    nc.vector.tensor_copy(osb[:], o_ps[:])
                    nc.scalar.dma_start(
                        out[m * 128:(m + 1) * 128, icol:icol + 512], osb[:])
                    yield

            def drain(gen):
                for _ in gen:
                    pass

            def interleave(gen_a, gen_b, ratio_a=1):
                alive = [gen_a, gen_b]
                while alive:
                    for g in list(alive):
                        steps = ratio_a if g is gen_a else 1
                        for _ in range(steps):
                            try:
                                next(g)
                            except StopIteration:
                                if g in alive:
                                    alive.remove(g)
                                break

            # ---- pipeline ----
            # All attention (and its ag_in staging writes) is emitted before
            # the first collective: Tile's cumulative DMA-lane semaphores
            # make any DMA consumer emitted after a collective wait for it,
            # so collectives must trail the compute that should overlap
            # them. The gpsimd queue dispatches each exchange as soon as its
            # staging writes land, giving the start cascade: (0,0) mid-B,
            # (0,1) end-B, (1,0) mid-C, (1,1) end-C.
            drain(qkv_blk(0))
            interleave(att_ib(0), qkv_blk(1))
            drain(att_ib(1))
            exchange(0, 0)
            exchange(0, 1)
            drain(oproj_blk(0))
            exchange(1, 0)
            exchange(1, 1)
            drain(oproj_blk(1))

    nc.compile()
    _NC_CACHE["nc"] = nc
    return nc


def _host_prep(x, Wq, Wk, Wv, Wo, head_scale):
    bf = ml_dtypes.bfloat16

    hs = np.asarray(head_scale).reshape(16)
    wo_s = (np.asarray(Wo) * np.repeat(hs, DH)[:, None]).astype(np.float32)

    def ktile(w):  # [2048, M] -> [128, 16, M]
        m = w.shape[1]
        return np.ascontiguousarray(
            w.reshape(KT, 128, m).transpose(1, 0, 2)).astype(bf)

    inv_freq = (1.0 / (10000.0 ** (np.arange(0, DH, 2, dtype=np.float64) / DH)))
    freqs = np.arange(N, dtype=np.float64)[:, None] * inv_freq[None, :]
    emb = np.concatenate([freqs, freqs], axis=-1)  # [N, 128]
    cosT = np.ascontiguousarray(np.cos(emb).T).astype(bf)
    sinT = np.sin(emb).T
    sign = np.where(np.arange(DH) < 64, -1.0, 1.0)[:, None]
    sinT = np.ascontiguousarray(sinT * sign).astype(bf)

    p = np.arange(128)[:, None]
    c = np.arange(512)[None, :]
    masks = [(c >= p + 128 * r).astype(np.float32) for r in range(4)]
    mask = np.concatenate(masks, axis=1).astype(bf)  # [128, 2048]

    x = np.asarray(x)
    xts = [np.ascontiguousarray(x[b].T).astype(bf) for b in range(B)]
    Wq = np.asarray(Wq)
    Wk = np.asarray(Wk)
    Wv = np.asarray(Wv)

    # half-gather k-tile order: [own h0-3, partner h0-3, own h4-7, ...]
    # expressed in global head indices per group g.
    def wo_ktile(w):  # [2048, M] -> k-tiles reordered to half-gather order
        kt = ktile(w)  # [128, 16, M] in global head order
        order = [0, 1, 2, 3, 8, 9, 10, 11, 4, 5, 6, 7, 12, 13, 14, 15]
        return np.ascontiguousarray(kt[:, order, :])

    in_maps = []
    for core in range(N_CORES):
        g = core % 2
        wq_k = ktile(Wq[:, g * 1024:(g + 1) * 1024])  # [128, 16, 1024]
        wq_h = np.ascontiguousarray(
            wq_k.reshape(128, KT, NQH, 128).transpose(0, 2, 1, 3))
        in_maps.append({
            "xt": xts[core // 2],
            "wq": wq_h,
            "wk": ktile(Wk[:, g * 256:(g + 1) * 256]),
            "wv": ktile(Wv[:, g * 256:(g + 1) * 256]),
            "wo": wo_ktile(wo_s[:, g * 1024:(g + 1) * 1024]),
            "cost": cosT,
            "sint": sinT,
            "mask": mask,
        })
    return in_maps


def kernel(x, Wq, Wk, Wv, Wo, head_scale, _run_kwargs=None):
    nc = build_nc()
    in_maps = _host_prep(x, Wq, Wk, Wv, Wo, head_scale)
    res = run_bass_kernel_spmd(
        nc, in_maps, core_ids=list(range(N_CORES)), **(_run_kwargs or {})
    )
    outs = []
    for b in range(B):
        o0 = res.results[2 * b]["out"].astype(np.float32)      # [1024, 1024]
        o1 = res.results[2 * b + 1]["out"].astype(np.float32)
        outs.append(np.concatenate([o0, o1], axis=0).T)         # [1024, 2048]
    full = np.ascontiguousarray(np.stack(outs, axis=0))
    if _run_kwargs:
        kernel.last_results = res
    return full


# revision 3
# speedup vs baseline: 1.0303x; 1.0303x over previous
"""Trainium2 8-core kernel for causal GQA attention (nn_Attention_90967407329949).

Distribution: 2-way tensor-parallel over head groups x 4-way data-parallel over
batch. Core c = (b = c//2, g = c%2) computes batch b with q heads g*8..g*8+7
and kv heads 2g, 2g+1 (no KV-projection redundancy). After attention, the
pair {2b, 2b+1} exchanges attention outputs via two 2-rank AllGathers (one per
512-token block); each core computes the output-projection column slice
out[:, g*1024:(g+1)*1024] for its batch. Host concatenates slices.

Causal wavefront pipeline per core (emission order = engine queue order):
  A: QKV projections for token block 0
  B: attention on query block 0 (needs only keys 0..511) interleaved with
     QKV projections for token block 1
  C: exchange #1 dispatched; attention on query block 1
  D: output projection block 0 (exchange #1 lands mid-C); exchange #2
  E: output projection block 1

All matmuls bf16 with fp32 PSUM. Softmax skips the running max (|s|max ~ 7).
The denominator reciprocal is broadcast across partitions with a K=1 TensorE
matmul (ones_row^T @ recip) instead of gpsimd.partition_broadcast, so the
GpSimd queue carries only the two collectives and never stalls attention.
The output projection reads the full gathered attention (own half included)
from ag_out so the SPMD program needs no per-core row offsets.

Layouts (feature dim on SBUF partitions):
  xt   [2048, 1024]   x[b]^T, bf16; on-chip [128, 16, 1024] k-tiles
  wq   [128, 16, 1024] Wq k-tiles for the 8 q heads of group g
  wk/wv[128, 16, 256]  k-tiles for the 2 kv heads of group g
  wo   [128, 16, 1024] (head_scale-folded) Wo k-tiles, column slice g
  cost/sint [128, 1024] rotary tables transposed; sint sign-folded
  mask [128, 2048]     4 causal masks for the 4 diagonal offsets
  out  [1024, 1024]    (out @ Wo)^T column slice, bf16 (host upcasts)
"""

import numpy as np
import ml_dtypes

import concourse.bacc as bacc
import concourse.mybir as mybir
import concourse.tile as tile
from concourse.bass_utils import run_bass_kernel_spmd

BF16 = mybir.dt.bfloat16
F32 = mybir.dt.float32

N_CORES = 8
B = 4
N = 1024           # sequence length per batch (per-core tokens)
D = 2048           # model dim
DH = 128           # head dim
KT = D // 128      # 16 contraction k-tiles
NQH = 8            # q heads per core
SCALE = 1.0 / np.sqrt(DH)

_NC_CACHE = {}


def build_nc():
    if "nc" in _NC_CACHE:
        return _NC_CACHE["nc"]
    nc = bacc.Bacc("TRN2", target_bir_lowering=False, debug=False,
                   num_devices=N_CORES)

    xt = nc.dram_tensor("xt", [D, N], BF16, kind="ExternalInput")
    wq = nc.dram_tensor("wq", [128, NQH, KT, 128], BF16, kind="ExternalInput")
    wk = nc.dram_tensor("wk", [128, KT, 256], BF16, kind="ExternalInput")
    wv = nc.dram_tensor("wv", [128, KT, 256], BF16, kind="ExternalInput")
    wo = nc.dram_tensor("wo", [128, KT, 1024], BF16, kind="ExternalInput")
    cost = nc.dram_tensor("cost", [128, N], BF16, kind="ExternalInput")
    sint = nc.dram_tensor("sint", [128, N], BF16, kind="ExternalInput")
    mask = nc.dram_tensor("mask", [128, 2048], BF16, kind="ExternalInput")
    grow = nc.dram_tensor("grow", [1, 256], BF16, kind="ExternalInput")
    out = nc.dram_tensor("out", [1024, N], BF16, kind="ExternalOutput")

    # Pair-exchange buffers: one per (token block, head half) so each
    # collective can start as soon as its 4 heads finish (tensor-granular
    # deps). The exchange is a 2-rank AllReduce(add) over [1024, 512]: rank
    # slot 0 rows hold group-0's heads 4q..4q+3, slot 1 group-1's. Each core
    # writes its attention into BOTH slots multiplied by a per-core 0/1 mask
    # (grow0/grow1), so the sum reconstructs the concatenation. AllReduce is
    # used instead of AllGather because its output bytes are produced by the
    # receiver's datapath from the partner's packets, so collective
    # completion structurally implies remote-data arrival (the 2-rank
    # AllGather push path was observed to signal completion before arrival
    # under core skew, corrupting the output nondeterministically).
    pair_groups = [[0, 1], [2, 3], [4, 5], [6, 7]]
    ag_in = [[nc.dram_tensor(f"ag_in{p}_{q}", [1024, 512], BF16)
              for q in range(2)] for p in range(2)]
    ag_out = [[nc.dram_tensor(f"ag_out{p}_{q}", [1024, 512], BF16)
               for q in range(2)] for p in range(2)]

    with tile.TileContext(nc) as tc:
        with (
            tc.tile_pool(name="const", bufs=1) as constp,
            tc.tile_pool(name="persist", bufs=1) as persist,
            tc.tile_pool(name="qkraw", bufs=3) as qkrawp,
            tc.tile_pool(name="rope", bufs=2) as ropep,
            tc.tile_pool(name="attp", bufs=2) as attp,
            tc.tile_pool(name="ep", bufs=3) as ep,
            tc.tile_pool(name="etmpp", bufs=1) as etmpp,
            tc.tile_pool(name="recipp", bufs=2) as recipp,
            tc.tile_pool(name="rbcp", bufs=2) as rbcp,
            tc.tile_pool(name="gp", bufs=2) as gp,
            tc.tile_pool(name="oobp", bufs=2) as oobp,
            tc.tile_pool(name="psacc", bufs=2, space="PSUM") as psacc,
            tc.tile_pool(name="pss", bufs=2, space="PSUM") as pss,
            tc.tile_pool(name="psu", bufs=2, space="PSUM") as psu,
            tc.tile_pool(name="pssum", bufs=1, space="PSUM") as pssum,
            tc.tile_pool(name="psb", bufs=1, space="PSUM") as psb,
        ):
            # ---- constants / persistent ----
            wq_sb = constp.tile([128, NQH, KT, 128], BF16)
            wk_sb = constp.tile([128, KT, 256], BF16)
            wv_sb = constp.tile([128, KT, 256], BF16)
            wo_sb = constp.tile([128, KT, 1024], BF16)
            cos_sb = constp.tile([128, N], BF16)
            sin_sb = constp.tile([128, N], BF16)
            mask_sb = constp.tile([128, 2048], BF16)
            ones_sb = constp.tile([128, 1], BF16)
            grow_sb = constp.tile([1, 256], BF16)
            xf = persist.tile([128, KT, N], BF16)

            xt_r = xt.rearrange("(t p) n -> p t n", p=128)

            # Bulk loads ride the gpsimd (SWDGE) and scalar rings so the
            # sync ring stays lean for small latency-critical transfers
            # (rope rotations, v transposes, exchange staging). First-use
            # order: K-proj needs wk + xf block 0, V-proj wv, then wq heads.
            nc.gpsimd.dma_start(wk_sb[:, 0:8], wk[:, 0:8])
            nc.scalar.dma_start(wv_sb[:, 0:8], wv[:, 0:8])
            nc.gpsimd.dma_start(xf[:, 0:8, 0:512], xt_r[:, 0:8, 0:512])
            nc.scalar.dma_start(xf[:, 8:16, 0:512], xt_r[:, 8:16, 0:512])
            nc.gpsimd.dma_start(wk_sb[:, 8:16], wk[:, 8:16])
            nc.scalar.dma_start(wv_sb[:, 8:16], wv[:, 8:16])
            nc.sync.dma_start(cos_sb[:], cost[:])
            nc.sync.dma_start(sin_sb[:], sint[:])
            for h in range(NQH):
                ring = nc.gpsimd if h < 4 else nc.scalar
                ring.dma_start(wq_sb[:, h], wq[:, h])
            nc.sync.dma_start(mask_sb[:], mask[:])
            nc.vector.memset(ones_sb[:], 1.0)
            nc.sync.dma_start(grow_sb[:], grow[:])

            def late_consts():
                # xf block 1, then wo (needed last)
                nc.gpsimd.dma_start(xf[:, 0:8, 512:1024],
                                    xt_r[:, 0:8, 512:1024])
                nc.scalar.dma_start(xf[:, 8:16, 512:1024],
                                    xt_r[:, 8:16, 512:1024])
                nc.gpsimd.dma_start(wo_sb[:], wo[:])

            # ---- persistent per-core QKV (RoPE'd, transposed layouts) ----
            q_sb = [persist.tile([128, N], BF16, name=f"q{h}_sb")
                    for h in range(NQH)]
            k_sb = [persist.tile([128, N], BF16, name=f"k{kv}_sb")
                    for kv in range(2)]
            v_sb = [persist.tile([128, N], BF16, name=f"v{kv}_sb")
                    for kv in range(2)]

            def rope_chunk(raw, dst, nb):
                """RoPE 512 positions (block nb) from raw into dst."""
                c0 = nb * 512
                rot = ropep.tile([128, 512], BF16, tag="rot")
                nc.sync.dma_start(rot[0:64, :], raw[64:128, :])
                nc.sync.dma_start(rot[64:128, :], raw[0:64, :])
                t1 = ropep.tile([128, 512], BF16, tag="t1")
                nc.vector.tensor_mul(t1[:], raw[:], cos_sb[:, c0:c0 + 512])
                t2 = ropep.tile([128, 512], BF16, tag="t2")
                nc.vector.tensor_mul(t2[:], rot[:], sin_sb[:, c0:c0 + 512])
                nc.vector.tensor_add(dst[:, c0:c0 + 512], t1[:], t2[:])

            def qkv_blk(nb):
                """Projections + RoPE for token block nb (512 tokens)."""
                col0 = nb * 512
                if nb == 1:
                    late_consts()
                holder = {}

                def accum(w_sb, msl, name):
                    ps = psacc.tile([128, 512], F32, tag="psacc", name=name)
                    for k0 in range(0, KT, 4):
                        for kt in range(k0, k0 + 4):
                            nc.tensor.matmul(
                                ps[:], w_sb[:, kt, msl],
                                xf[:, kt, col0:col0 + 512],
                                start=(kt == 0), stop=(kt == KT - 1))
                        yield
                    holder["ps"] = ps

                # K, V first (attention consumes them head-by-head)
                for kv in range(2):
                    msl = slice(kv * 128, (kv + 1) * 128)
                    yield from accum(wk_sb, msl, f"k_ps_{nb}_{kv}")
                    kraw = qkrawp.tile([128, 512], BF16, tag="raw")
                    nc.scalar.activation(kraw[:], holder["ps"][:],
                                         mybir.ActivationFunctionType.Copy)
                    rope_chunk(kraw, k_sb[kv], nb)
                    yield
                    yield from accum(wv_sb, msl, f"v_ps_{nb}_{kv}")
                    vraw = qkrawp.tile([128, 512], BF16, tag="raw")
                    nc.scalar.activation(vraw[:], holder["ps"][:],
                                         mybir.ActivationFunctionType.Copy)
                    for i in range(4):
                        ring = nc.sync if i % 2 == 0 else nc.scalar
                        ring.dma_start_transpose(
                            v_sb[kv][:, col0 + i * 128:col0 + (i + 1) * 128],
                            vraw[:, i * 128:(i + 1) * 128])
                    yield
                for h in range(NQH):
                    yield from accum(wq_sb[:, h], slice(0, 128),
                                     f"q_ps_{nb}_{h}")
                    qraw = qkrawp.tile([128, 512], BF16, tag="raw")
                    nc.scalar.activation(qraw[:], holder["ps"][:],
                                         mybir.ActivationFunctionType.Copy)
                    rope_chunk(qraw, q_sb[h], nb)
                    yield

            def att_ib(ib):
                """Attention for query block ib, all heads; yields per j-tile."""
                icol = ib * 512
                cnt = 4 * ib + 4
                for h in range(NQH):
                    qh = q_sb[h]
                    kv = h // 4
                    u_ps = psu.tile([128, 512], F32, tag="psu",
                                    name=f"u_ps_{ib}_{h}")
                    sum_ps = pssum.tile([1, 512], F32, tag="pssum",
                                        name=f"sum_ps_{ib}_{h}")

                    def c_lo(jt):
                        r = jt - 4 * ib
                        return 128 * r if r > 0 else 0

                    def s_mm(jt):
                        s_ps = pss.tile([128, 512], F32, tag="pss",
                                        name=f"s_ps_{ib}_{h}_{jt}")
                        jcol = jt * 128
                        c0 = c_lo(jt)
                        nc.tensor.matmul(
                            s_ps[:, c0:512], k_sb[kv][:, jcol:jcol + 128],
                            qh[:, icol + c0:icol + 512],
                            start=True, stop=True)
                        return s_ps

                    def e_of(jt, s_ps):
                        r = jt - 4 * ib
                        c0 = c_lo(jt)
                        e = ep.tile([128, 512], BF16, tag="e",
                                    name=f"e_{ib}_{h}_{jt}")
                        if r >= 0:  # diagonal tile: mask after exp
                            etmp = etmpp.tile([128, 512], BF16, tag="etmp")
                            nc.scalar.activation(
                                etmp[:, c0:512], s_ps[:, c0:512],
                                mybir.ActivationFunctionType.Exp, scale=SCALE)
                            nc.vector.tensor_mul(
                                e[:, c0:512], etmp[:, c0:512],
                                mask_sb[:, r * 512 + c0:(r + 1) * 512])
                        else:
                            nc.scalar.activation(
                                e[:], s_ps[:],
                                mybir.ActivationFunctionType.Exp, scale=SCALE)
                        return e

                    s_tiles = {0: s_mm(0), 1: s_mm(1)}
                    for jt in range(cnt):
                        e = e_of(jt, s_tiles.pop(jt))
                        if jt + 2 < cnt:
                            s_tiles[jt + 2] = s_mm(jt + 2)
                        c0 = c_lo(jt)
                        nc.tensor.matmul(
                            u_ps[:, c0:512],
                            v_sb[kv][:, jt * 128:(jt + 1) * 128], e[:, c0:512],
                            start=(jt == 0), stop=(jt == cnt - 1),
                            skip_group_check=True)
                        nc.tensor.matmul(
                            sum_ps[:, c0:512], ones_sb[:], e[:, c0:512],
                            start=(jt == 0), stop=(jt == cnt - 1),
                            skip_group_check=True)
                        yield
                    recip_f = recipp.tile([1, 512], F32, tag="recipf")
                    nc.vector.reciprocal_approx_fast(out=recip_f[:],
                                                     in_=sum_ps[:])
                    recip = recipp.tile([1, 512], BF16, tag="recip")
                    nc.vector.tensor_copy(recip[:], recip_f[:])
                    q, hq = h // 4, h % 4
                    for sl in range(2):
                        rbc_ps = psb.tile([128, 512], F32, tag="psb",
                                          name=f"rbc_ps_{ib}_{h}_{sl}")
                        nc.tensor.matmul(
                            rbc_ps[:], grow_sb[0:1, sl * 128:(sl + 1) * 128],
                            recip[:], start=True, stop=True)
                        rbc = rbcp.tile([128, 512], BF16, tag=f"rbc{sl}")
                        nc.scalar.activation(rbc[:], rbc_ps[:],
                                             mybir.ActivationFunctionType.Copy)
                        atile = attp.tile([128, 512], BF16, tag=f"att{sl}",
                                          name=f"att_{ib}_{h}_{sl}")
                        nc.vector.tensor_mul(atile[:], u_ps[:], rbc[:])
                        yield
                        ring = nc.sync if sl == 0 else nc.scalar
                        ring.dma_start(
                            ag_in[ib][q][sl * 512 + hq * 128:
                                         sl * 512 + (hq + 1) * 128, :],
                            atile[:])

            def exchange(ib, q):
                nc.gpsimd.collective_compute(
                    "AllReduce", mybir.AluOpType.add,
                    replica_groups=pair_groups,
                    ins=[ag_in[ib][q][:].opt()],
                    outs=[ag_out[ib][q][:].opt()])

            ag_out_r = [[t.rearrange("(t p) n -> p t n", p=128) for t in row]
                        for row in ag_out]

            def oproj_blk(ib):
                """Output projection for token block ib from the full gather."""
                icol = ib * 512
                n0, n1 = 8, 8
                g0 = gp.tile([128, n0, 512], BF16, tag="g0", name=f"g0_{ib}")
                g1 = gp.tile([128, n1, 512], BF16, tag="g1", name=f"g1_{ib}")
                nc.sync.dma_start(g0[:], ag_out_r[ib][0][:, 0:n0, :])
                nc.scalar.dma_start(g1[:], ag_out_r[ib][1][:, 0:n1, :])

                def gsrc(kt):
                    return g0[:, kt, :] if kt < n0 else g1[:, kt - n0, :]

                for m in range(8):
                    o_ps = psacc.tile([128, 512], F32, tag="psacc",
                                      name=f"o_ps_{ib}_{m}")
                    for k0 in range(0, KT, 4):
                        for kt in range(k0, k0 + 4):
                            nc.tensor.matmul(
                                o_ps[:], wo_sb[:, kt, m * 128:(m + 1) * 128],
                                gsrc(kt), start=(kt == 0),
                                stop=(kt == KT - 1))
                        yield
                    osb = oobp.tile([128, 512], BF16, tag="osb",
                                    name=f"osb_{ib}_{m}")
                    nc.vector.tensor_copy(osb[:], o_ps[:])
                    nc.scalar.dma_start(
                        out[m * 128:(m + 1) * 128, icol:icol + 512], osb[:])
                    yield

            def drain(gen):
                for _ in gen:
                    pass

            def interleave(gen_a, gen_b, ratio_a=1):
                alive = [gen_a, gen_b]
                while alive:
                    for g in list(alive):
                        steps = ratio_a if g is gen_a else 1
                        for _ in range(steps):
                            try:
                                next(g)
                            except StopIteration:
                                if g in alive:
                                    alive.remove(g)
                                break

            # ---- pipeline ----
            # All attention (and its ag_in staging writes) is emitted before
            # the first collective: Tile's cumulative DMA-lane semaphores
            # make any DMA consumer emitted after a collective wait for it,
            # so collectives must trail the compute that should overlap
            # them. The gpsimd queue dispatches each exchange as soon as its
            # staging writes land, giving the start cascade: (0,0) mid-B,
            # (0,1) end-B, (1,0) mid-C, (1,1) end-C.
            drain(qkv_blk(0))
            interleave(att_ib(0), qkv_blk(1))
            drain(att_ib(1))
            exchange(0, 0)
            exchange(0, 1)
            drain(oproj_blk(0))
            exchange(1, 0)
            exchange(1, 1)
            drain(oproj_blk(1))

    nc.compile()
    _NC_CACHE["nc"] = nc
    return nc


def _host_prep(x, Wq, Wk, Wv, Wo, head_scale):
    bf = ml_dtypes.bfloat16

    hs = np.asarray(head_scale).reshape(16)
    wo_s = (np.asarray(Wo) * np.repeat(hs, DH)[:, None]).astype(np.float32)

    def ktile(w):  # [2048, M] -> [128, 16, M]
        m = w.shape[1]
        return np.ascontiguousarray(
            w.reshape(KT, 128, m).transpose(1, 0, 2)).astype(bf)

    inv_freq = (1.0 / (10000.0 ** (np.arange(0, DH, 2, dtype=np.float64) / DH)))
    freqs = np.arange(N, dtype=np.float64)[:, None] * inv_freq[None, :]
    emb = np.concatenate([freqs, freqs], axis=-1)  # [N, 128]
    cosT = np.ascontiguousarray(np.cos(emb).T).astype(bf)
    sinT = np.sin(emb).T
    sign = np.where(np.arange(DH) < 64, -1.0, 1.0)[:, None]
    sinT = np.ascontiguousarray(sinT * sign).astype(bf)

    p = np.arange(128)[:, None]
    c = np.arange(512)[None, :]
    masks = [(c >= p + 128 * r).astype(np.float32) for r in range(4)]
    mask = np.concatenate(masks, axis=1).astype(bf)  # [128, 2048]

    x = np.asarray(x)
    xts = [np.ascontiguousarray(x[b].T).astype(bf) for b in range(B)]
    Wq = np.asarray(Wq)
    Wk = np.asarray(Wk)
    Wv = np.asarray(Wv)

    # half-gather k-tile order: [own h0-3, partner h0-3, own h4-7, ...]
    # expressed in global head indices per group g.
    def wo_ktile(w):  # [2048, M] -> k-tiles reordered to half-gather order
        kt = ktile(w)  # [128, 16, M] in global head order
        order = [0, 1, 2, 3, 8, 9, 10, 11, 4, 5, 6, 7, 12, 13, 14, 15]
        return np.ascontiguousarray(kt[:, order, :])

    in_maps = []
    for core in range(N_CORES):
        g = core % 2
        wq_k = ktile(Wq[:, g * 1024:(g + 1) * 1024])  # [128, 16, 1024]
        wq_h = np.ascontiguousarray(
            wq_k.reshape(128, KT, NQH, 128).transpose(0, 2, 1, 3))
        in_maps.append({
            "xt": xts[core // 2],
            "wq": wq_h,
            "wk": ktile(Wk[:, g * 256:(g + 1) * 256]),
            "wv": ktile(Wv[:, g * 256:(g + 1) * 256]),
            "wo": wo_ktile(wo_s[:, g * 1024:(g + 1) * 1024]),
            "cost": cosT,
            "sint": sinT,
            "mask": mask,
            "grow": np.concatenate(
                [np.full((1, 128), 1.0 - g), np.full((1, 128), float(g))],
                axis=1).astype(bf),
        })
    return in_maps


def kernel(x, Wq, Wk, Wv, Wo, head_scale, _run_kwargs=None):
    nc = build_nc()
    in_maps = _host_prep(x, Wq, Wk, Wv, Wo, head_scale)
    res = run_bass_kernel_spmd(
        nc, in_maps, core_ids=list(range(N_CORES)), **(_run_kwargs or {})
    )
    outs = []
    for b in range(B):
        o0 = res.results[2 * b]["out"].astype(np.float32)      # [1024, 1024]
        o1 = res.results[2 * b + 1]["out"].astype(np.float32)
        outs.append(np.concatenate([o0, o1], axis=0).T)         # [1024, 2048]
    full = np.ascontiguousarray(np.stack(outs, axis=0))
    if _run_kwargs:
        kernel.last_results = res
    return full
